# revision 1
# baseline (speedup 1.0000x reference)
"""MADPSNet MoE-routing kernel for 8 Trainium2 NeuronCores.

The reference computes every expert on the full stacked input and then
gathers one expert per agent.  The routing indices (laac_shallow /
laac_deep) are host-visible numpy values, so we do the routing on the
host: per agent we select the 4 weight matrices of its chosen experts
and run only the selected chain

    x[2048,256] @ W1[256,512] -> relu -> @ W2[512,256] -> relu
                -> @ W3[256,512] -> relu -> @ W4[512,128] (+bias)

One agent per NeuronCore (A == 8 == n_cores), no collectives.

Layout: everything feature-major on chip (features on the 128
partitions, batch on the free dim).  The host pre-packs

    x   [128, 4096]     col = bt*1024 + k*512 + b  (bt-major batch tiles)
    wN  [128, K/128*M]  col = (k*mc + m)*128 + j   (k-chunk-major)
    bias[128, 11]       col j = 128-chunk j of [b1(4) b2(2) b3(4) b4(1)]

so every DMA is a large contiguous transfer, issued in compute-need
order on the two HWDGE queues (x on sync/SP, weights on scalar/ACT).
Matmuls run in float32r (full PE rate for moving dim >= 256, ~fp32
accuracy), PSUM accumulates fp32, bias+relu runs split across ScalarE
and VectorE with a fixed engine per destination tile, and the layers
are emitted as a (bt + 2*layer) diagonal wavefront so the in-order PE
queue always has ready work while L1 waits on x DMAs.  A few warm-up
matmuls on a zeroed scratch tile keep the PE busy from kernel start so
the HAM clock un-throttles (1.2 -> 2.4 GHz) before the real work
arrives.  The kernel returns out^T [128, 2048] per core; the host
transposes back.
"""

import os

import numpy as np

import concourse.bass as bass
import concourse.mybir as mybir
from concourse import bacc
from concourse.bass_utils import run_bass_kernel_spmd
from concourse.tile import TileContext

A, B, S = 8, 2048, 256
H1, H2, D1, D2 = 512, 256, 512, 128
P = 128
BT = 512            # batch tile (psum bank: 512 fp32)
NBT = B // BT

_DT_MAP = {
    "f32": mybir.dt.float32,
    "f32r": mybir.dt.float32r,
    "bf16": mybir.dt.bfloat16,
}

# layer: (k_chunks, m_chunks, bias col offset, relu?)
_LAYERS = [
    (S // P, H1 // P, 0, True),    # L1: 256 -> 512
    (H1 // P, H2 // P, 4, True),   # L2: 512 -> 256
    (H2 // P, D1 // P, 6, True),   # L3: 256 -> 512
    (D1 // P, D2 // P, 10, False), # L4: 512 -> 128
]


def _build(dt_name: str, add_bias: bool, warm: int) -> bass.Bass:
    dt = _DT_MAP[dt_name]
    f32 = mybir.dt.float32
    nc = bacc.Bacc(None, target_bir_lowering=False, debug=False)

    x_d = nc.dram_tensor("x", [P, (S // P) * B], dt, kind="ExternalInput")
    w_ds = [
        nc.dram_tensor("w1", [P, (S // P) * H1], dt, kind="ExternalInput"),
        nc.dram_tensor("w2", [P, (H1 // P) * H2], dt, kind="ExternalInput"),
        nc.dram_tensor("w3", [P, (H2 // P) * D1], dt, kind="ExternalInput"),
        nc.dram_tensor("w4", [P, (D1 // P) * D2], dt, kind="ExternalInput"),
    ]
    b_d = (
        nc.dram_tensor("bias", [P, 11], f32, kind="ExternalInput")
        if add_bias
        else None
    )
    out_d = nc.dram_tensor("out", [D2, B], f32, kind="ExternalOutput")

    with TileContext(nc) as tc:
        with (
            tc.tile_pool(name="persist", bufs=1) as pp,
            tc.tile_pool(name="psum", bufs=8, space="PSUM") as psp,
        ):
            xt = pp.tile([P, (S // P) * B], dt, tag="xt", name="xt")
            wts = [
                pp.tile(
                    [P, w_ds[i].shape[1]], dt, tag=f"w{i}", name=f"w{i}_sb"
                )
                for i in range(4)
            ]
            bti = (
                pp.tile([P, 11], f32, tag="bias", name="bias_sb")
                if add_bias
                else None
            )
            scr = (
                pp.tile([P, 2], f32, tag="scr", name="scr") if add_bias else None
            )
            acts = [
                [
                    pp.tile([P, B], dt, tag=f"a{li}_{i}", name=f"a{li}_{i}")
                    for i in range(n)
                ]
                for li, n in [(1, H1 // P), (2, H2 // P), (3, D1 // P)]
            ]
            acts.append([pp.tile([P, B], f32, tag="ot", name="ot")])

            # ---- PE warm-up: matmuls on a scratch tile so the HAM clock
            # gate opens (~3.4us of PE busy) before real data lands.  The
            # scratch is zeroed so no stray NaNs sit in the PE datapath.
            if warm > 0:
                wdt = f32 if dt == mybir.dt.float32r else dt
                wsb = pp.tile([P, BT], wdt, tag="wsb", name="wsb")
                nc.gpsimd.memset(wsb[:], 0.0)
                wps = psp.tile([P, BT], f32, tag="ps", name="wps")
                lhs = wsb[:, 0:P]
                rhs = wsb[:]
                if dt == mybir.dt.float32r:
                    lhs = lhs.bitcast(dt)
                    rhs = rhs.bitcast(dt)
                for _ in range(warm):
                    nc.tensor.matmul(wps[:], lhs, rhs, start=True, stop=True)

            # ---- input DMAs: x per batch-tile on the sync HWDGE queue,
            # weights whole on the scalar (ACT) HWDGE queue — two parallel
            # descriptor streams.  x is host-packed bt-major (col =
            # bt*2*BT + k*BT + b) so each per-bt transfer is a contiguous
            # 4KB-per-partition chunk.
            kx = S // P

            def dma_x(eng, bt, k):
                sl = slice((bt * kx + k) * BT, (bt * kx + k + 1) * BT)
                eng.dma_start(xt[:, sl], x_d[:, sl])

            # Supply paced to the wavefront's demand, balanced across the
            # two HWDGE queues (SP=sync, ACT=scalar): bt0/bt1/bt2 split
            # per k-half so each lands as L1 reaches it.
            nc.scalar.dma_start(wts[0][:, 0:512], w_ds[0][:, 0:512])
            nc.scalar.dma_start(wts[0][:, 512:1024], w_ds[0][:, 512:1024])
            dma_x(nc.scalar, 1, 1)
            nc.scalar.dma_start(wts[1][:], w_ds[1][:])
            nc.scalar.dma_start(wts[3][:], w_ds[3][:])
            for bt, k in ((0, 0), (0, 1), (1, 0), (2, 0), (2, 1), (3, 0), (3, 1)):
                dma_x(nc.sync, bt, k)
            nc.sync.dma_start(wts[2][:], w_ds[2][:])
            if add_bias:
                nc.sync.dma_start(bti[:], b_d[:])
                # advance ACT/DVE engine clocks past the bias DMA so the
                # real post-matmul ops carry a single (PE) wait each — the
                # AC/DVE instruction structs have one wait slot.
                nc.scalar.copy(scr[:, 0:1], bti[:, 0:1])
                nc.vector.tensor_copy(scr[:, 1:2], bti[:, 0:1])

            # ---- the 4-layer chain, emitted as a (bt + 2*layer) diagonal
            # wavefront: the PE's in-order queue then always has ready
            # later-layer work to chew while L1 waits on x DMAs.
            def x_rhs(k, bt):
                return xt[:, (bt * kx + k) * BT : (bt * kx + k + 1) * BT]

            sched = sorted(
                ((bt + 2 * li, -li, bt) for li in range(4) for bt in range(NBT))
            )
            for _, nli, bt in sched:
                li = -nli
                kc, mc, boff, relu = _LAYERS[li]
                wt = wts[li]
                dsts = acts[li]
                srcs = acts[li - 1] if li > 0 else None
                if li == 0:
                    # k-outer for every L1 batch-tile: each k sweep needs
                    # only one 256KB x chunk + half of w1 in SBUF, so the
                    # supply-paced phase runs with fine-grained waits
                    pss = [
                        psp.tile([P, BT], f32, tag="ps", name=f"ps_l0_{bt}_{m}")
                        for m in range(mc)
                    ]
                    for k in range(kc):
                        for m in range(mc):
                            nc.tensor.matmul(
                                pss[m][:],
                                wt[:, (k * mc + m) * P : (k * mc + m + 1) * P],
                                x_rhs(k, bt),
                                start=(k == 0),
                                stop=(k == kc - 1),
                            )
                else:
                    pss = None
                if True:
                    for m in range(mc):
                        # fixed engine per dst tile: one writer per tile
                        use_act = (li < 3) and (m < mc // 2 or mc == 1)
                        if pss is not None:
                            ps = pss[m]
                        else:
                            ps = psp.tile([P, BT], f32, tag="ps", name="ps")
                            for k in range(kc):
                                rhs = (
                                    x_rhs(k, bt)
                                    if li == 0
                                    else srcs[k][:, bt * BT : (bt + 1) * BT]
                                )
                                nc.tensor.matmul(
                                    ps[:],
                                    wt[:, (k * mc + m) * P : (k * mc + m + 1) * P],
                                    rhs,
                                    start=(k == 0),
                                    stop=(k == kc - 1),
                                )
                        dst = dsts[m][:, bt * BT : (bt + 1) * BT]
                        if add_bias:
                            bias_ap = bti[:, boff + m : boff + m + 1]
                            if use_act:
                                func = (
                                    mybir.ActivationFunctionType.Relu
                                    if relu
                                    else mybir.ActivationFunctionType.Identity
                                )
                                nc.scalar.activation(
                                    dst, ps[:], func, bias=bias_ap
                                )
                            elif relu:
                                nc.vector.tensor_scalar(
                                    dst,
                                    ps[:],
                                    bias_ap,
                                    0.0,
                                    mybir.AluOpType.add,
                                    mybir.AluOpType.max,
                                )
                            else:
                                nc.vector.tensor_scalar_add(dst, ps[:], bias_ap)
                        elif use_act:
                            func = (
                                mybir.ActivationFunctionType.Relu
                                if relu
                                else mybir.ActivationFunctionType.Copy
                            )
                            nc.scalar.activation(dst, ps[:], func)
                        elif relu:
                            nc.vector.tensor_scalar_max(dst, ps[:], 0.0)
                        elif li == 3 and bt == NBT - 1:
                            # quarter the last copy so the final out-DMA
                            # chunks are small and start early
                            q = BT // 4
                            for j in range(4):
                                nc.vector.tensor_copy(
                                    dst[:, j * q : (j + 1) * q],
                                    ps[:, j * q : (j + 1) * q],
                                )
                        else:
                            nc.vector.tensor_copy(dst, ps[:])
                    if li == 3:
                        ot = acts[3][0]
                        if bt < NBT - 1:
                            eng = nc.sync if bt % 2 == 0 else nc.scalar
                            eng.dma_start(
                                out_d[:, bt * BT : (bt + 1) * BT],
                                ot[:, bt * BT : (bt + 1) * BT],
                            )
                        else:
                            # last tile: quarter across both queues to
                            # shorten the final drain
                            q = BT // 4
                            o = bt * BT
                            for j in range(4):
                                eng = nc.sync if j % 2 == 0 else nc.scalar
                                eng.dma_start(
                                    out_d[:, o + j * q : o + (j + 1) * q],
                                    ot[:, o + j * q : o + (j + 1) * q],
                                )
    nc.compile()
    return nc


_BUILT: dict[tuple, bass.Bass] = {}


def _cfg():
    dt_name = os.environ.get("MADPS_DT", "f32r")
    warm = int(os.environ.get("MADPS_WARM", "10"))
    return dt_name, warm


def _get_nc(dt_name: str, add_bias: bool, warm: int) -> bass.Bass:
    key = (dt_name, add_bias, warm)
    if key not in _BUILT:
        _BUILT[key] = _build(dt_name, add_bias, warm)
    return _BUILT[key]


def _np_dt(dt_name: str):
    if dt_name == "bf16":
        import ml_dtypes

        return ml_dtypes.bfloat16
    return np.float32


def _packw(w: np.ndarray, np_dt) -> np.ndarray:
    """[K, M] -> [128, (K/128)*M], k-chunk-major: col (k*mc + m)*128 + j."""
    k, m = w.shape
    kc = k // P
    return np.ascontiguousarray(
        w.reshape(kc, P, m).transpose(1, 0, 2).reshape(P, -1).astype(np_dt)
    )


def _prepare(inputs, dt_name):
    """Returns (add_bias, in_maps) for run_bass_kernel_spmd."""
    np_dt = _np_dt(dt_name)

    x = np.asarray(inputs["inputs"], dtype=np.float32)
    sel_s = np.asarray(inputs["laac_shallow"]).reshape(-1).astype(np.int64)
    sel_d = np.asarray(inputs["laac_deep"]).reshape(-1).astype(np.int64)
    Ws1 = np.asarray(inputs["Ws1"], dtype=np.float32)
    Ws2 = np.asarray(inputs["Ws2"], dtype=np.float32)
    Wd1 = np.asarray(inputs["Wd1"], dtype=np.float32)
    Wd2 = np.asarray(inputs["Wd2"], dtype=np.float32)
    bs1 = np.asarray(inputs["bs1"], dtype=np.float32)
    bs2 = np.asarray(inputs["bs2"], dtype=np.float32)
    bd1 = np.asarray(inputs["bd1"], dtype=np.float32)
    bd2 = np.asarray(inputs["bd2"], dtype=np.float32)

    add_bias = any(
        float(np.abs(b).max()) != 0.0 for b in (bs1, bs2, bd1, bd2)
    )

    in_maps = []
    for a in range(A):
        es, ed = int(sel_s[a]), int(sel_d[a])
        # bt-major packing: col = bt*(S//P)*BT + k*BT + b
        xp = np.ascontiguousarray(
            x[a]
            .reshape(NBT, BT, S // P, P)
            .transpose(3, 0, 2, 1)
            .reshape(P, -1)
            .astype(np_dt)
        )
        m = {
            "x": xp,
            "w1": _packw(Ws1[es], np_dt),
            "w2": _packw(Ws2[es], np_dt),
            "w3": _packw(Wd1[ed], np_dt),
            "w4": _packw(Wd2[ed], np_dt),
        }
        if add_bias:
            bias_cols = np.concatenate([bs1[es], bs2[es], bd1[ed], bd2[ed]])
            m["bias"] = np.ascontiguousarray(
                bias_cols.reshape(11, P).T, dtype=np.float32
            )
        in_maps.append(m)
    return add_bias, in_maps


def kernel(**inputs) -> np.ndarray:
    dt_name, warm = _cfg()
    add_bias, in_maps = _prepare(inputs, dt_name)
    nc = _get_nc(dt_name, add_bias, warm)
    res = run_bass_kernel_spmd(nc, in_maps, list(range(A)))
    out = np.stack([np.asarray(res.results[a]["out"]).T for a in range(A)])
    return np.ascontiguousarray(out.astype(np.float32))



# revision 7
# speedup vs baseline: 1.0714x; 1.0714x over previous
"""MADPSNet MoE-routing kernel for 8 Trainium2 NeuronCores.

The reference computes every expert on the full stacked input and then
gathers one expert per agent.  The routing indices (laac_shallow /
laac_deep) are host-visible numpy values, so we do the routing on the
host: per agent we select the 4 weight matrices of its chosen experts
and run only the selected chain

    x[2048,256] @ W1[256,512] -> relu -> @ W2[512,256] -> relu
                -> @ W3[256,512] -> relu -> @ W4[512,128] (+bias)

One agent per NeuronCore (A == 8 == n_cores), no collectives.

Layout: feature-major on chip (features on the 128 partitions, batch on
the free dim), everything bf16 except the fp32 PSUM accumulators (the
harness tolerance is 2e-2; bf16 end-to-end lands ~1e-3).  bf16 halves
the HBM traffic and runs matmuls at full PE rate with fast weight load,
vs the ~1.27x slower fp32 HIGH-mode pairs the fp32 path emits.

The batch is processed as 2 super-tiles of 1024 (pairs of 512-column
PSUM banks): each [128,1024] PSUM pair tile is filled by two matmul
accumulation groups (same output chunk m, two adjacent batch tiles) and
drained by a single ACT/DVE op, which amortizes the ~300-400ns fixed
PSUM-access cost per consumer op and keeps the per-partition bias
scalar unique per op.  Activations are stored m-chunk-major so a pair
drain is one contiguous 1024-column write.

DMAs are spread over three queues in compute-need order: sync (HWDGE)
takes x(bt0), w1, x(bt1); scalar (HWDGE) takes w2, x(bt2) behind the
hoisted ACT table load; gpsimd (SWDGE) takes w3, x(bt3), w4.  A few
warm-up matmuls on a zeroed scratch tile keep the PE busy from the end
of the framework preamble so the HAM clock gate opens (1.2 -> 2.4 GHz)
before the real work arrives.  The kernel returns out^T [128, 2048]
bf16 per core; the host transposes and upcasts.
"""

import os

import numpy as np

import concourse.bass as bass
import concourse.mybir as mybir
from concourse import bacc
from concourse.bass_utils import run_bass_kernel_spmd
from concourse.tile import TileContext

A, B, S = 8, 2048, 256
H1, H2, D1, D2 = 512, 256, 512, 128
P = 128
BT = 512            # batch tile (psum bank: 512 fp32)
NBT = B // BT
NBP = NBT // 2      # batch super-tiles (pairs)

_DT_MAP = {
    "f32": mybir.dt.float32,
    "f32r": mybir.dt.float32r,
    "bf16": mybir.dt.bfloat16,
}

# layer: (k_chunks, m_chunks, bias col offset, relu?)
_LAYERS = [
    (S // P, H1 // P, 0, True),    # L1: 256 -> 512
    (H1 // P, H2 // P, 4, True),   # L2: 512 -> 256
    (H2 // P, D1 // P, 6, True),   # L3: 256 -> 512
    (D1 // P, D2 // P, 10, False), # L4: 512 -> 128
]


def _build(
    dt_name: str, add_bias: bool, warm: int, paird: bool, swdge: bool
) -> bass.Bass:
    dt = _DT_MAP[dt_name]
    f32 = mybir.dt.float32
    nc = bacc.Bacc(None, target_bir_lowering=False, debug=False)

    kx = S // P
    x_d = nc.dram_tensor("x", [P, kx * B], dt, kind="ExternalInput")
    w_ds = [
        nc.dram_tensor("w1", [P, (S // P) * H1], dt, kind="ExternalInput"),
        nc.dram_tensor("w2", [P, (H1 // P) * H2], dt, kind="ExternalInput"),
        nc.dram_tensor("w3", [P, (H2 // P) * D1], dt, kind="ExternalInput"),
        nc.dram_tensor("w4", [P, (D1 // P) * D2], dt, kind="ExternalInput"),
    ]
    b_d = (
        nc.dram_tensor("bias", [P, 11], f32, kind="ExternalInput")
        if add_bias
        else None
    )
    out_d = nc.dram_tensor("out", [D2, B], dt, kind="ExternalOutput")

    with TileContext(nc) as tc:
        with (
            tc.tile_pool(name="persist", bufs=1) as pp,
            tc.tile_pool(name="psum", bufs=4, space="PSUM") as psp,
        ):
            xt = pp.tile([P, kx * B], dt, tag="xt", name="xt")
            wts = [
                pp.tile(
                    [P, w_ds[i].shape[1]], dt, tag=f"w{i}", name=f"w{i}_sb"
                )
                for i in range(4)
            ]
            bti = (
                pp.tile([P, 11], f32, tag="bias", name="bias_sb")
                if add_bias
                else None
            )
            scr = (
                pp.tile([P, 2], f32, tag="scr", name="scr") if add_bias else None
            )
            # activations, m-chunk-major: col = (m*NBT + bt)*BT + b
            acts = [
                pp.tile([P, n * B], dt, tag=f"a{li}", name=f"a{li}")
                for li, n in [(1, H1 // P), (2, H2 // P), (3, D1 // P)]
            ]
            ot = pp.tile([P, B], dt, tag="ot", name="ot")

            # ---- input DMAs in compute-need order across three queues.
            # x is host-packed bt-major (col = bt*kx*BT + k*BT + b) so each
            # per-bt transfer is one contiguous 2KB-per-partition chunk.
            def dma_x(eng, bt):
                sl = slice(bt * kx * BT, (bt + 1) * kx * BT)
                eng.dma_start(xt[:, sl], x_d[:, sl])

            dma_x(nc.sync, 0)
            nc.sync.dma_start(wts[0][:], w_ds[0][:])
            dma_x(nc.sync, 1)
            nc.scalar.dma_start(wts[1][:], w_ds[1][:])
            dma_x(nc.scalar, 2)

            # ---- PE warm-up on a zeroed scratch tile so the HAM clock
            # gate opens before real data lands.
            wsb = pp.tile([P, BT], dt, tag="wsb", name="wsb")
            wps = psp.tile([P, 2 * BT], f32, tag="pp", name="wps")
            if warm > 0:
                nc.gpsimd.memset(wsb[:], 0.0)
                for _ in range(warm):
                    nc.tensor.matmul(
                        wps[:, 0:BT], wsb[:, 0:P], wsb[:], start=True, stop=True
                    )

            dq = nc.gpsimd if swdge else nc.sync
            dq2 = nc.gpsimd if swdge else nc.scalar
            dq.dma_start(wts[2][:], w_ds[2][:])
            dq2.dma_start(xt[:, 3 * kx * BT : 4 * kx * BT],
                          x_d[:, 3 * kx * BT : 4 * kx * BT])
            dq.dma_start(wts[3][:], w_ds[3][:])
            if add_bias:
                dq2.dma_start(bti[:], b_d[:])
                # advance ACT/DVE engine clocks past the bias DMA so the
                # real post-matmul ops carry a single (PE) wait each.
                nc.scalar.copy(scr[:, 0:1], bti[:, 0:1])
                nc.vector.tensor_copy(scr[:, 1:2], bti[:, 0:1])

            # ---- the 4-layer chain over 2 batch super-tiles, bf16
            # matmuls accumulating into [128,1024] two-bank PSUM pairs.
            def rhs(li, k, bt):
                if li == 0:
                    return xt[:, (bt * kx + k) * BT : (bt * kx + k + 1) * BT]
                src = acts[li - 1]
                return src[:, (k * NBT + bt) * BT : (k * NBT + bt + 1) * BT]

            ndrain = 0

            def drain(ps_ap, dst, boff_m, relu, split):
                """PSUM -> SBUF with bias+relu; `split` halves the op
                across both engines (for the final tile)."""
                nonlocal ndrain
                parts = 2 if split else 1
                w = ps_ap.shape[1] // parts
                for j in range(parts):
                    use_act = (ndrain % 2 == 0) if not split else (j == 0)
                    ndrain += 1
                    src = ps_ap[:, j * w : (j + 1) * w]
                    d = dst[:, j * w : (j + 1) * w]
                    if add_bias:
                        bias_ap = bti[:, boff_m : boff_m + 1]
                        if use_act:
                            func = (
                                mybir.ActivationFunctionType.Relu
                                if relu
                                else mybir.ActivationFunctionType.Identity
                            )
                            nc.scalar.activation(d, src, func, bias=bias_ap)
                        elif relu:
                            nc.vector.tensor_scalar(
                                d,
                                src,
                                bias_ap,
                                0.0,
                                mybir.AluOpType.add,
                                mybir.AluOpType.max,
                            )
                        else:
                            nc.vector.tensor_scalar_add(d, src, bias_ap)
                    elif use_act:
                        func = (
                            mybir.ActivationFunctionType.Relu
                            if relu
                            else mybir.ActivationFunctionType.Copy
                        )
                        nc.scalar.activation(d, src, func)
                    elif relu:
                        nc.vector.tensor_scalar_max(d, src, 0.0)
                    else:
                        nc.vector.tensor_copy(d, src)

            for li in range(4):
                kc, mc, boff, relu = _LAYERS[li]
                wt = wts[li]
                for btp in range(NBP):
                    last_block = li == 3 and btp == NBP - 1
                    pairs = [
                        psp.tile(
                            [P, 2 * BT], f32, tag="pp", name=f"ps{li}_{btp}_{m}"
                        )
                        for m in range(mc)
                    ]
                    for half in range(2):
                        bt = 2 * btp + half
                        for m in range(mc):
                            half_ap = pairs[m][:, half * BT : (half + 1) * BT]
                            for k in range(kc):
                                nc.tensor.matmul(
                                    half_ap,
                                    wt[:, (k * mc + m) * P : (k * mc + m + 1) * P],
                                    rhs(li, k, bt),
                                    start=(k == 0),
                                    stop=(k == kc - 1),
                                )
                            if half == 1:
                                if li < 3:
                                    dst = acts[li][
                                        :,
                                        (m * NBT + 2 * btp) * BT :
                                        (m * NBT + 2 * btp + 2) * BT,
                                    ]
                                else:
                                    dst = ot[:, 2 * btp * BT : (2 * btp + 2) * BT]
                                drain(
                                    pairs[m][:],
                                    dst,
                                    boff + m,
                                    relu,
                                    split=last_block or not paird,
                                )
                    if li == 3:
                        # ship each super-tile as soon as it is drained;
                        # final tile split across both HWDGE queues.
                        o = 2 * btp * BT
                        if not last_block:
                            nc.sync.dma_start(
                                out_d[:, o : o + 2 * BT], ot[:, o : o + 2 * BT]
                            )
                        else:
                            nc.sync.dma_start(
                                out_d[:, o : o + BT], ot[:, o : o + BT]
                            )
                            nc.scalar.dma_start(
                                out_d[:, o + BT : o + 2 * BT],
                                ot[:, o + BT : o + 2 * BT],
                            )
    nc.compile()
    return nc


_BUILT: dict[tuple, bass.Bass] = {}


def _cfg():
    dt_name = os.environ.get("MADPS_DT", "bf16")
    warm = int(os.environ.get("MADPS_WARM", "7"))
    return dt_name, warm


def _feat(name: str, default: str = "1") -> bool:
    return os.environ.get(name, default) == "1"


def _get_nc(dt_name: str, add_bias: bool, warm: int) -> bass.Bass:
    paird = _feat("MADPS_PAIRD")
    swdge = _feat("MADPS_SWDGE")
    key = (dt_name, add_bias, warm, paird, swdge)
    if key not in _BUILT:
        _BUILT[key] = _build(dt_name, add_bias, warm, paird, swdge)
    return _BUILT[key]


def _np_dt(dt_name: str):
    if dt_name == "bf16":
        import ml_dtypes

        return ml_dtypes.bfloat16
    return np.float32


def _packw(w: np.ndarray, np_dt) -> np.ndarray:
    """[K, M] -> [128, (K/128)*M], k-chunk-major: col (k*mc + m)*128 + j."""
    k, m = w.shape
    kc = k // P
    return np.ascontiguousarray(
        w.reshape(kc, P, m).transpose(1, 0, 2).reshape(P, -1).astype(np_dt)
    )


def _prepare(inputs, dt_name):
    """Returns (add_bias, in_maps) for run_bass_kernel_spmd."""
    np_dt = _np_dt(dt_name)

    x = np.asarray(inputs["inputs"], dtype=np.float32)
    sel_s = np.asarray(inputs["laac_shallow"]).reshape(-1).astype(np.int64)
    sel_d = np.asarray(inputs["laac_deep"]).reshape(-1).astype(np.int64)
    Ws1 = np.asarray(inputs["Ws1"], dtype=np.float32)
    Ws2 = np.asarray(inputs["Ws2"], dtype=np.float32)
    Wd1 = np.asarray(inputs["Wd1"], dtype=np.float32)
    Wd2 = np.asarray(inputs["Wd2"], dtype=np.float32)
    bs1 = np.asarray(inputs["bs1"], dtype=np.float32)
    bs2 = np.asarray(inputs["bs2"], dtype=np.float32)
    bd1 = np.asarray(inputs["bd1"], dtype=np.float32)
    bd2 = np.asarray(inputs["bd2"], dtype=np.float32)

    add_bias = any(
        float(np.abs(b).max()) != 0.0 for b in (bs1, bs2, bd1, bd2)
    )

    in_maps = []
    for a in range(A):
        es, ed = int(sel_s[a]), int(sel_d[a])
        # bt-major packing: col = bt*(S//P)*BT + k*BT + b
        xp = np.ascontiguousarray(
            x[a]
            .reshape(NBT, BT, S // P, P)
            .transpose(3, 0, 2, 1)
            .reshape(P, -1)
            .astype(np_dt)
        )
        m = {
            "x": xp,
            "w1": _packw(Ws1[es], np_dt),
            "w2": _packw(Ws2[es], np_dt),
            "w3": _packw(Wd1[ed], np_dt),
            "w4": _packw(Wd2[ed], np_dt),
        }
        if add_bias:
            bias_cols = np.concatenate([bs1[es], bs2[es], bd1[ed], bd2[ed]])
            m["bias"] = np.ascontiguousarray(
                bias_cols.reshape(11, P).T, dtype=np.float32
            )
        in_maps.append(m)
    return add_bias, in_maps


def kernel(**inputs) -> np.ndarray:
    dt_name, warm = _cfg()
    add_bias, in_maps = _prepare(inputs, dt_name)
    nc = _get_nc(dt_name, add_bias, warm)
    res = run_bass_kernel_spmd(nc, in_maps, list(range(A)))
    out = np.stack(
        [np.asarray(res.results[a]["out"]).astype(np.float32).T for a in range(A)]
    )
    return np.ascontiguousarray(out)


# revision 10
# speedup vs baseline: 1.0889x; 1.0164x over previous
"""MADPSNet MoE-routing kernel for 8 Trainium2 NeuronCores.

The reference computes every expert on the full stacked input and then
gathers one expert per agent.  The routing indices (laac_shallow /
laac_deep) are host-visible numpy values, so we do the routing on the
host: per agent we select the 4 weight matrices of its chosen experts
and run only the selected chain

    x[2048,256] @ W1[256,512] -> relu -> @ W2[512,256] -> relu
                -> @ W3[256,512] -> relu -> @ W4[512,128] (+bias)

One agent per NeuronCore (A == 8 == n_cores), no collectives.

Layout: feature-major on chip (features on the 128 partitions, batch on
the free dim), everything bf16 except the fp32 PSUM accumulators (the
harness tolerance is 2e-2; bf16 end-to-end lands ~1e-3).  bf16 halves
the HBM traffic and runs matmuls at full PE rate with fast weight load,
vs the ~1.27x slower fp32 HIGH-mode pairs the fp32 path emits.

The batch is processed as 2 super-tiles of 1024 (pairs of 512-column
PSUM banks): each [128,1024] PSUM pair tile is filled by two matmul
accumulation groups (same output chunk m, two adjacent batch tiles) and
drained by a single ACT/DVE op, which amortizes the ~300-400ns fixed
PSUM-access cost per consumer op and keeps the per-partition bias
scalar unique per op.  Activations are stored m-chunk-major so a pair
drain is one contiguous 1024-column write.

DMAs are spread over three queues in compute-need order: sync (HWDGE)
takes x(bt0), w1, x(bt1); scalar (HWDGE) takes w2, x(bt2) behind the
hoisted ACT table load; gpsimd (SWDGE) takes w3, x(bt3), w4.  A few
warm-up matmuls on a zeroed scratch tile keep the PE busy from the end
of the framework preamble so the HAM clock gate opens (1.2 -> 2.4 GHz)
before the real work arrives.  The kernel returns out^T [128, 2048]
bf16 per core; the host transposes and upcasts.
"""

import os

import numpy as np

import concourse.bass as bass
import concourse.mybir as mybir
from concourse import bacc
from concourse.bass_utils import run_bass_kernel_spmd
from concourse.tile import TileContext

A, B, S = 8, 2048, 256
H1, H2, D1, D2 = 512, 256, 512, 128
P = 128
BT = 512            # batch tile (psum bank: 512 fp32)
NBT = B // BT
NBP = NBT // 2      # batch super-tiles (pairs)

_DT_MAP = {
    "f32": mybir.dt.float32,
    "f32r": mybir.dt.float32r,
    "bf16": mybir.dt.bfloat16,
}

# layer: (k_chunks, m_chunks, bias col offset, relu?)
_LAYERS = [
    (S // P, H1 // P, 0, True),    # L1: 256 -> 512
    (H1 // P, H2 // P, 4, True),   # L2: 512 -> 256
    (H2 // P, D1 // P, 6, True),   # L3: 256 -> 512
    (D1 // P, D2 // P, 10, False), # L4: 512 -> 128
]


def _build(
    dt_name: str, add_bias: bool, warm: int, paird: bool, swdge: bool
) -> bass.Bass:
    dt = _DT_MAP[dt_name]
    f32 = mybir.dt.float32
    nc = bacc.Bacc(None, target_bir_lowering=False, debug=False)

    kx = S // P
    x_d = nc.dram_tensor("x", [P, kx * B], dt, kind="ExternalInput")
    w_ds = [
        nc.dram_tensor("w1", [P, (S // P) * H1], dt, kind="ExternalInput"),
        nc.dram_tensor("w2", [P, (H1 // P) * H2], dt, kind="ExternalInput"),
        nc.dram_tensor("w3", [P, (H2 // P) * D1], dt, kind="ExternalInput"),
        nc.dram_tensor("w4", [P, (D1 // P) * D2], dt, kind="ExternalInput"),
    ]
    b_d = (
        nc.dram_tensor("bias", [P, 11], f32, kind="ExternalInput")
        if add_bias
        else None
    )
    out_d = nc.dram_tensor("out", [D2, B], dt, kind="ExternalOutput")

    with TileContext(nc) as tc:
        with (
            tc.tile_pool(name="persist", bufs=1) as pp,
            tc.tile_pool(name="psum", bufs=4, space="PSUM") as psp,
        ):
            xt = pp.tile([P, kx * B], dt, tag="xt", name="xt")
            wts = [
                pp.tile(
                    [P, w_ds[i].shape[1]], dt, tag=f"w{i}", name=f"w{i}_sb"
                )
                for i in range(4)
            ]
            bti = (
                pp.tile([P, 11], f32, tag="bias", name="bias_sb")
                if add_bias
                else None
            )
            scr = (
                pp.tile([P, 2], f32, tag="scr", name="scr") if add_bias else None
            )
            # activations, m-chunk-major: col = (m*NBT + bt)*BT + b
            acts = [
                pp.tile([P, n * B], dt, tag=f"a{li}", name=f"a{li}")
                for li, n in [(1, H1 // P), (2, H2 // P), (3, D1 // P)]
            ]
            ot = pp.tile([P, B], dt, tag="ot", name="ot")

            # ---- input DMAs.  Phase 1 (sync queue, immediate): the L1
            # critical path x(bt0), w1, x(bt1) gets all 16 SDMA engines to
            # itself.  Phase 2 (scalar + gpsimd) is gated on w1 landing via
            # tiny copies reading the w1 tile, so its packets don't
            # round-robin-steal engines from phase 1 and delay the first
            # real matmul.
            gts = pp.tile([P, 2], dt, tag="gts", name="gts")

            def dma_x(eng, bt):
                sl = slice(bt * kx * BT, (bt + 1) * kx * BT)
                eng.dma_start(xt[:, sl], x_d[:, sl])

            dma_x(nc.sync, 0)
            nc.sync.dma_start(wts[0][:], w_ds[0][:])
            dma_x(nc.sync, 1)

            # ---- PE warm-up on a zeroed scratch tile so the HAM clock
            # gate opens before real data lands.
            wsb = pp.tile([P, BT], dt, tag="wsb", name="wsb")
            wps = psp.tile([P, 2 * BT], f32, tag="pp", name="wps")
            if warm > 0:
                nc.gpsimd.memset(wsb[:], 0.0)
                for _ in range(warm):
                    nc.tensor.matmul(
                        wps[:, 0:BT], wsb[:, 0:P], wsb[:], start=True, stop=True
                    )

            gate = wts[0][:, wts[0].shape[1] - 1 :]
            nc.scalar.copy(gts[:, 0:1], gate)
            if add_bias:
                nc.scalar.dma_start(bti[:], b_d[:])
            dma_x(nc.scalar, 2)
            nc.scalar.dma_start(wts[1][:], w_ds[1][:])

            dq = nc.gpsimd if swdge else nc.scalar
            if swdge:
                nc.gpsimd.tensor_copy(gts[:, 1:2], gate)
            dma_x(dq, 3)
            dq.dma_start(wts[2][:], w_ds[2][:])
            dq.dma_start(wts[3][:], w_ds[3][:])
            if add_bias:
                # advance ACT/DVE engine clocks past the bias DMA so the
                # real post-matmul ops carry a single (PE) wait each.
                nc.scalar.copy(scr[:, 0:1], bti[:, 0:1])
                nc.vector.tensor_copy(scr[:, 1:2], bti[:, 0:1])

            # ---- the 4-layer chain over 2 batch super-tiles, bf16
            # matmuls accumulating into [128,1024] two-bank PSUM pairs.
            def rhs(li, k, bt):
                if li == 0:
                    return xt[:, (bt * kx + k) * BT : (bt * kx + k + 1) * BT]
                src = acts[li - 1]
                return src[:, (k * NBT + bt) * BT : (k * NBT + bt + 1) * BT]

            ndrain = 0

            def drain(ps_ap, dst, boff_m, relu, split):
                """PSUM -> SBUF with bias+relu; `split` halves the op
                across both engines (for the final tile)."""
                nonlocal ndrain
                parts = 2 if split else 1
                w = ps_ap.shape[1] // parts
                for j in range(parts):
                    use_act = (ndrain % 2 == 1) if not split else (j == 0)
                    ndrain += 1
                    src = ps_ap[:, j * w : (j + 1) * w]
                    d = dst[:, j * w : (j + 1) * w]
                    if add_bias:
                        bias_ap = bti[:, boff_m : boff_m + 1]
                        if use_act:
                            func = (
                                mybir.ActivationFunctionType.Relu
                                if relu
                                else mybir.ActivationFunctionType.Identity
                            )
                            nc.scalar.activation(d, src, func, bias=bias_ap)
                        elif relu:
                            nc.vector.tensor_scalar(
                                d,
                                src,
                                bias_ap,
                                0.0,
                                mybir.AluOpType.add,
                                mybir.AluOpType.max,
                            )
                        else:
                            nc.vector.tensor_scalar_add(d, src, bias_ap)
                    elif use_act:
                        func = (
                            mybir.ActivationFunctionType.Relu
                            if relu
                            else mybir.ActivationFunctionType.Copy
                        )
                        nc.scalar.activation(d, src, func)
                    elif relu:
                        nc.vector.tensor_scalar_max(d, src, 0.0)
                    else:
                        nc.vector.tensor_copy(d, src)

            for li in range(4):
                kc, mc, boff, relu = _LAYERS[li]
                wt = wts[li]
                for btp in range(NBP):
                    pairs = [
                        psp.tile(
                            [P, 2 * BT], f32, tag="pp", name=f"ps{li}_{btp}_{m}"
                        )
                        for m in range(mc)
                    ]
                    for half in range(2):
                        bt = 2 * btp + half
                        for m in range(mc):
                            half_ap = pairs[m][:, half * BT : (half + 1) * BT]
                            for k in range(kc):
                                nc.tensor.matmul(
                                    half_ap,
                                    wt[:, (k * mc + m) * P : (k * mc + m + 1) * P],
                                    rhs(li, k, bt),
                                    start=(k == 0),
                                    stop=(k == kc - 1),
                                )
                            if li == 3:
                                # drain + ship each 512-col half as soon as
                                # its batch tile's accumulation finishes, so
                                # the final out-DMA chain starts early.
                                dst = ot[:, bt * BT : (bt + 1) * BT]
                                drain(half_ap, dst, boff + m, relu, split=False)
                                eng = nc.sync if half == 0 else nc.scalar
                                eng.dma_start(
                                    out_d[:, bt * BT : (bt + 1) * BT], dst
                                )
                            elif half == 1:
                                dst = acts[li][
                                    :,
                                    (m * NBT + 2 * btp) * BT :
                                    (m * NBT + 2 * btp + 2) * BT,
                                ]
                                drain(
                                    pairs[m][:],
                                    dst,
                                    boff + m,
                                    relu,
                                    split=not paird,
                                )
    nc.compile()
    return nc


_BUILT: dict[tuple, bass.Bass] = {}


def _cfg():
    dt_name = os.environ.get("MADPS_DT", "bf16")
    warm = int(os.environ.get("MADPS_WARM", "8"))
    return dt_name, warm


def _feat(name: str, default: str = "1") -> bool:
    return os.environ.get(name, default) == "1"


def _get_nc(dt_name: str, add_bias: bool, warm: int) -> bass.Bass:
    paird = _feat("MADPS_PAIRD")
    swdge = _feat("MADPS_SWDGE")
    key = (dt_name, add_bias, warm, paird, swdge)
    if key not in _BUILT:
        _BUILT[key] = _build(dt_name, add_bias, warm, paird, swdge)
    return _BUILT[key]


def _np_dt(dt_name: str):
    if dt_name == "bf16":
        import ml_dtypes

        return ml_dtypes.bfloat16
    return np.float32


def _packw(w: np.ndarray, np_dt) -> np.ndarray:
    """[K, M] -> [128, (K/128)*M], k-chunk-major: col (k*mc + m)*128 + j."""
    k, m = w.shape
    kc = k // P
    return np.ascontiguousarray(
        w.reshape(kc, P, m).transpose(1, 0, 2).reshape(P, -1).astype(np_dt)
    )


def _prepare(inputs, dt_name):
    """Returns (add_bias, in_maps) for run_bass_kernel_spmd."""
    np_dt = _np_dt(dt_name)

    x = np.asarray(inputs["inputs"], dtype=np.float32)
    sel_s = np.asarray(inputs["laac_shallow"]).reshape(-1).astype(np.int64)
    sel_d = np.asarray(inputs["laac_deep"]).reshape(-1).astype(np.int64)
    Ws1 = np.asarray(inputs["Ws1"], dtype=np.float32)
    Ws2 = np.asarray(inputs["Ws2"], dtype=np.float32)
    Wd1 = np.asarray(inputs["Wd1"], dtype=np.float32)
    Wd2 = np.asarray(inputs["Wd2"], dtype=np.float32)
    bs1 = np.asarray(inputs["bs1"], dtype=np.float32)
    bs2 = np.asarray(inputs["bs2"], dtype=np.float32)
    bd1 = np.asarray(inputs["bd1"], dtype=np.float32)
    bd2 = np.asarray(inputs["bd2"], dtype=np.float32)

    add_bias = any(
        float(np.abs(b).max()) != 0.0 for b in (bs1, bs2, bd1, bd2)
    )

    in_maps = []
    for a in range(A):
        es, ed = int(sel_s[a]), int(sel_d[a])
        # bt-major packing: col = bt*(S//P)*BT + k*BT + b
        xp = np.ascontiguousarray(
            x[a]
            .reshape(NBT, BT, S // P, P)
            .transpose(3, 0, 2, 1)
            .reshape(P, -1)
            .astype(np_dt)
        )
        m = {
            "x": xp,
            "w1": _packw(Ws1[es], np_dt),
            "w2": _packw(Ws2[es], np_dt),
            "w3": _packw(Wd1[ed], np_dt),
            "w4": _packw(Wd2[ed], np_dt),
        }
        if add_bias:
            bias_cols = np.concatenate([bs1[es], bs2[es], bd1[ed], bd2[ed]])
            m["bias"] = np.ascontiguousarray(
                bias_cols.reshape(11, P).T, dtype=np.float32
            )
        in_maps.append(m)
    return add_bias, in_maps


def kernel(**inputs) -> np.ndarray:
    dt_name, warm = _cfg()
    add_bias, in_maps = _prepare(inputs, dt_name)
    nc = _get_nc(dt_name, add_bias, warm)
    res = run_bass_kernel_spmd(nc, in_maps, list(range(A)))
    out = np.stack(
        [np.asarray(res.results[a]["out"]).astype(np.float32).T for a in range(A)]
    )
    return np.ascontiguousarray(out)
